# revision 12
# baseline (speedup 1.0000x reference)
"""MoE (8 experts, top-2) Trainium2 kernel — FFN-sliced expert parallel, bf16.

Strategy:
  - Host: router (softmax + top-2 + renorm), dispatch tokens per expert.
  - Work unit = (expert, F-quarter): y_part = GELU(X_e @ W1[e][:, q]) @ W2[e][q, :]
    32 units assigned to 8 cores x 4 slots. Slot j (same capacity on every
    core) holds the 4 quarters of two experts (cores 0-3 expert A, 4-7
    expert B), with experts paired by sorted token count so slot capacity
    tracks the actual load: sum(caps) ~ 8288 token-slots/core instead of
    the 2304x4 = 9216 a whole-expert-per-core layout needs. Per-core PE
    floor = sum(caps) * 128 cycles.
  - All matmuls bf16 (fp32 PSUM accumulate): measured end-to-end rel err
    ~3.4e-3. bf16 also enables Fast Weight Load (fp32 weights disable FWL).
  - Host: sum the 4 F-quarter partials (f32), add b2, weighted combine.

Device loop per slot (cap tokens, token groups of 512, passes of 2 groups):
  mm1: ps1[f128, g] = sum_dt W1t[dt, ft].T @ Xt[dt, g]   (bf16)
  h[ft, g] = Gelu(ps1 + b1[ft])  -> SBUF bf16
  mm2: ps2[d128, g] = sum_ft W2t[ft, dt2].T @ h[ft, g]   (bf16)
  y[dt2, g] = copy(ps2) -> DRAM bf16 (partials; summed f32 on host)
"""

import numpy as np
import ml_dtypes

import concourse.bacc as bacc
import concourse.mybir as mybir
import concourse.tile as tile
from concourse.bass import ds, ts
from concourse.bass_utils import run_bass_kernel_spmd

P = 128
D_MODEL = 1024
D_FF = 4096
NUM_EXPERTS = 8
TOP_K = 2
NDT = D_MODEL // P   # 8 d-tiles
QF = D_FF // 4       # 1024: F-quarter width
NFQ = QF // P        # 8 f-tiles per quarter
NSLOTS = 4
GS = 512             # token group (matmul moving dim)
PASS_G = 2           # groups per psum pass (double-buffered)

f32 = mybir.dt.float32
bf16 = mybir.dt.bfloat16
BF16_NP = ml_dtypes.bfloat16
Y_DT = bf16  # y-partial output dtype (bf16 halves output DMA; rel err 3.8e-3 vs 3.4e-3)

_BUILT = {}


def _groups(cap):
    return [(o, min(GS, cap - o)) for o in range(0, cap, GS)]


def _passes(cap):
    gs = _groups(cap)
    return [gs[i : i + PASS_G] for i in range(0, len(gs), PASS_G)]


def _build(caps: tuple, repeats: int = 1):
    """Per-core module: NSLOTS independent quarter-FFN units, caps[j] tokens."""
    nc = bacc.Bacc(None, target_bir_lowering=False)

    xt_d, w1_d, w2_d, b1_d, y_d = [], [], [], [], []
    for j, cap in enumerate(caps):
        xt_d.append(
            nc.declare_dram_parameter(f"xt{j}", [P, NDT, cap], bf16, isOutput=False)
        )
        # w1 tile k = ft*NDT + dt ; w2 tile k = dt2*NFQ + ft
        w1_d.append(
            nc.declare_dram_parameter(f"w1_{j}", [P, NFQ, NDT, P], bf16, isOutput=False)
        )
        w2_d.append(
            nc.declare_dram_parameter(f"w2_{j}", [P, NDT, NFQ, P], bf16, isOutput=False)
        )
        b1_d.append(
            nc.declare_dram_parameter(f"b1_{j}", [P, NFQ], f32, isOutput=False)
        )
        y_d.append(
            nc.declare_dram_parameter(f"y{j}", [P, NDT, cap], Y_DT, isOutput=True)
        )

    with tile.TileContext(nc) as tc:
        with (
            tc.tile_pool(name="const", bufs=1) as const_pool,
            tc.tile_pool(name="xt", bufs=2) as xt_pool,
            tc.tile_pool(name="w1", bufs=2) as w1_pool,
            tc.tile_pool(name="w2", bufs=1) as w2_pool,
            tc.tile_pool(name="h", bufs=2) as h_pool,
            tc.tile_pool(name="yo", bufs=3) as y_pool,
            tc.tile_pool(name="ps1", bufs=2, space="PSUM") as ps1_pool,
            tc.tile_pool(name="ps2", bufs=2, space="PSUM") as ps2_pool,
        ):
            b1_sb = []
            for j in range(NSLOTS):
                t = const_pool.tile([P, NFQ], f32, name=f"b1sb{j}")
                nc.sync.dma_start(out=t[:], in_=b1_d[j][:])
                b1_sb.append(t)

            for j in [jj % NSLOTS for jj in range(NSLOTS * repeats)]:
                cap = caps[j]
                passes = _passes(cap)

                # x: one tile per d-tile so mm1 can start on the first slice
                x_sb = []
                for dt in range(NDT):
                    t = xt_pool.tile([P, cap], bf16, name=f"x{dt}")
                    nc.sync.dma_start(
                        out=t[:], in_=xt_d[j][:, ts(dt, 1)].squeeze()
                    )
                    x_sb.append(t)
                # weights: per-ft (w1) / per-dt2 (w2) tiles for fine deps
                w1_sb = []
                for ft in range(NFQ):
                    t = w1_pool.tile([P, NDT, P], bf16, name=f"w1t{ft}")
                    nc.sync.dma_start(out=t[:], in_=w1_d[j][:, ts(ft, 1)].squeeze())
                    w1_sb.append(t)
                w2_sb = []
                for dt2 in range(NDT):
                    t = w2_pool.tile([P, NFQ, P], bf16, name=f"w2t{dt2}")
                    nc.sync.dma_start(out=t[:], in_=w2_d[j][:, ts(dt2, 1)].squeeze())
                    w2_sb.append(t)

                # ---- mm1 + gelu -> h (bf16, per-ft tiles) ----
                h_sb = []
                for ft in range(NFQ):
                    h_t = h_pool.tile([P, cap], bf16, name=f"h{ft}")
                    for pgs in passes:
                        ps = [
                            ps1_pool.tile([P, g[1]], f32, name=f"ps1_{i}")
                            for i, g in enumerate(pgs)
                        ]
                        for dt in range(NDT):
                            for i, (go, gsz) in enumerate(pgs):
                                nc.tensor.matmul(
                                    ps[i][:],
                                    w1_sb[ft][:, ts(dt, 1)].squeeze(),
                                    x_sb[dt][:, ds(go, gsz)],
                                    start=(dt == 0),
                                    stop=(dt == NDT - 1),
                                    skip_group_check=True,
                                )
                        for i, (go, gsz) in enumerate(pgs):
                            nc.scalar.activation(
                                h_t[:, ds(go, gsz)],
                                ps[i][:],
                                mybir.ActivationFunctionType.Gelu,
                                bias=b1_sb[j][:, ts(ft, 1)],
                            )
                    h_sb.append(h_t)

                # ---- mm2 -> y (f32) ----
                for dt2 in range(NDT):
                    y_sb = y_pool.tile([P, cap], Y_DT, name="ysb")
                    for pgs in passes:
                        ps = [
                            ps2_pool.tile([P, g[1]], f32, name=f"ps2_{i}")
                            for i, g in enumerate(pgs)
                        ]
                        for ft in range(NFQ):
                            for i, (go, gsz) in enumerate(pgs):
                                nc.tensor.matmul(
                                    ps[i][:],
                                    w2_sb[dt2][:, ts(ft, 1)].squeeze(),
                                    h_sb[ft][:, ds(go, gsz)],
                                    start=(ft == 0),
                                    stop=(ft == NFQ - 1),
                                    skip_group_check=True,
                                )
                        for i, (go, gsz) in enumerate(pgs):
                            nc.vector.tensor_copy(
                                y_sb[:, ds(go, gsz)], ps[i][:]
                            )
                    nc.sync.dma_start(
                        out=y_d[j][:, ts(dt2, 1)].squeeze(), in_=y_sb[:]
                    )

    nc.compile()
    return nc


def _get_built(caps, repeats: int = 1):
    key = (tuple(caps), repeats)
    if key not in _BUILT:
        _BUILT[key] = _build(tuple(caps), repeats)
    return _BUILT[key]


def _route(x_flat, Wr, br):
    """Router: softmax over experts, top-2, renormalized. Pure numpy."""
    logits = x_flat.astype(np.float32) @ Wr.astype(np.float32) + br.astype(np.float32)
    m = logits.max(axis=-1, keepdims=True)
    p = np.exp(logits - m)
    p /= p.sum(axis=-1, keepdims=True)
    i0 = np.argmax(p, axis=-1)
    pm = p.copy()
    pm[np.arange(p.shape[0]), i0] = -np.inf
    i1 = np.argmax(pm, axis=-1)
    w0 = p[np.arange(p.shape[0]), i0]
    w1 = p[np.arange(p.shape[0]), i1]
    s = w0 + w1
    return i0, i1, w0 / s, w1 / s


def _pad8(n):
    return max(GS // 4, (n + 7) // 8 * 8)


def kernel(x, Wr, br, W1, b1, W2, b2, _run_kwargs=None):
    x = np.asarray(x)
    B, L, D = x.shape
    T = B * L
    x_flat = np.ascontiguousarray(x.reshape(T, D), dtype=np.float32)

    i0, i1, w0, w1c = _route(x_flat, Wr, br)

    rows_l, wts_l = [], []
    for e in range(NUM_EXPERTS):
        sel = (i0 == e) | (i1 == e)
        rows = np.nonzero(sel)[0]
        w = np.where(i0[rows] == e, w0[rows], w1c[rows]).astype(np.float32)
        rows_l.append(rows)
        wts_l.append(w)

    counts = np.array([len(r) for r in rows_l])
    order = np.argsort(-counts, kind="stable")
    # slot j holds experts (order[2j], order[2j+1]); cap = max of the pair
    pairs = [(int(order[2 * j]), int(order[2 * j + 1])) for j in range(NSLOTS)]
    caps = tuple(_pad8(max(counts[a], counts[b])) for a, b in pairs)
    nc = _get_built(caps)

    # Per-expert packed arrays (shared across the 4 quarter-units)
    xt_e, w1_e, w2_e, b1_e = {}, {}, {}, {}
    for j, (ea, eb) in enumerate(pairs):
        cap = caps[j]
        for e in (ea, eb):
            rows = rows_l[e]
            xe = np.zeros((cap, D_MODEL), dtype=np.float32)
            xe[: len(rows)] = x_flat[rows]
            # [cap, D] -> [D, cap] -> [NDT, P, cap] -> [P, NDT, cap]
            xt_e[e] = np.ascontiguousarray(
                xe.T.reshape(NDT, P, cap).transpose(1, 0, 2)
            ).astype(BF16_NP)
            w1f = np.asarray(W1[e], dtype=np.float32)  # [D, F]
            w2f = np.asarray(W2[e], dtype=np.float32)  # [F, D]
            b1f = np.asarray(b1[e], dtype=np.float32)  # [F]
            w1_e[e], w2_e[e], b1_e[e] = [], [], []
            for q in range(4):
                w1q = w1f[:, q * QF : (q + 1) * QF]  # [1024, 1024]
                # [NDT, P, NFQ, P] -> [P, NFQ, NDT, P]
                w1_e[e].append(
                    np.ascontiguousarray(
                        w1q.reshape(NDT, P, NFQ, P).transpose(1, 2, 0, 3)
                    ).astype(BF16_NP)
                )
                w2q = w2f[q * QF : (q + 1) * QF, :]  # [1024, 1024]
                # [NFQ, P, NDT, P] -> [P, NDT, NFQ, P]
                w2_e[e].append(
                    np.ascontiguousarray(
                        w2q.reshape(NFQ, P, NDT, P).transpose(1, 2, 0, 3)
                    ).astype(BF16_NP)
                )
                b1_e[e].append(
                    np.ascontiguousarray(
                        b1f[q * QF : (q + 1) * QF].reshape(NFQ, P).T
                    )
                )

    in_maps = []
    for c in range(NUM_EXPERTS):
        q = c % 4
        m = {}
        for j, (ea, eb) in enumerate(pairs):
            e = ea if c < 4 else eb
            m[f"xt{j}"] = xt_e[e]
            m[f"w1_{j}"] = w1_e[e][q]
            m[f"w2_{j}"] = w2_e[e][q]
            m[f"b1_{j}"] = b1_e[e][q]
        in_maps.append(m)

    kw = dict(_run_kwargs or {})
    res = run_bass_kernel_spmd(nc, in_maps, list(range(NUM_EXPERTS)), **kw)

    # Combine: sum 4 quarter-partials per expert, add b2, weighted scatter
    out = np.zeros((T, D_MODEL), dtype=np.float32)
    for j, (ea, eb) in enumerate(pairs):
        cap = caps[j]
        acc = {ea: None, eb: None}
        for c in range(NUM_EXPERTS):
            e = ea if c < 4 else eb
            part = np.asarray(res.results[c][f"y{j}"], dtype=np.float32)
            acc[e] = part if acc[e] is None else acc[e] + part
        for e in (ea, eb):
            rows = rows_l[e]
            ye = acc[e].transpose(1, 0, 2).reshape(D_MODEL, cap)  # [D, cap]
            ye = ye[:, : len(rows)].T + np.asarray(b2[e], dtype=np.float32)
            out[rows] += wts_l[e][:, None] * ye

    kernel._last_result = res
    kernel._last_in_maps = in_maps
    kernel._last_cap = caps
    return out.reshape(B, L, D_MODEL)


def make_bench_runner(nc, in_maps, n_cores=NUM_EXPERTS):
    """Device-resident repeat-execution runner for timing (mirrors
    bass2jax.run_bass_via_pjrt's multi-core path, but stages inputs on
    device once and creates donated zero outputs on-device)."""
    import jax
    import jax.numpy as jnp
    from jax.experimental.shard_map import shard_map
    from jax.sharding import Mesh, NamedSharding, PartitionSpec

    from concourse import bass2jax
    from concourse import mybir as _mybir

    bass2jax.install_neuronx_cc_hook()

    part_name = (
        nc.partition_id_tensor.name if nc.partition_id_tensor else None
    )
    in_names, out_names, out_avals = [], [], []
    for alloc in nc.m.functions[0].allocations:
        if not isinstance(alloc, _mybir.MemoryLocationSet):
            continue
        name = alloc.memorylocations[0].name
        if alloc.kind == "ExternalInput":
            if name != part_name:
                in_names.append(name)
        elif alloc.kind == "ExternalOutput":
            out_names.append(name)
            out_avals.append(
                jax.core.ShapedArray(
                    tuple(alloc.tensor_shape), _mybir.dt.np(alloc.dtype)
                )
            )
    n_params = len(in_names)
    all_in = in_names + out_names
    if part_name is not None:
        all_in = all_in + [part_name]

    def _body(*args):
        operands = list(args)
        if part_name is not None:
            operands.append(bass2jax.partition_id_tensor())
        outs = bass2jax._bass_exec_p.bind(
            *operands,
            out_avals=tuple(out_avals),
            in_names=tuple(all_in),
            out_names=tuple(out_names),
            lowering_input_output_aliases=(),
            sim_require_finite=True,
            sim_require_nnan=True,
            nc=nc,
        )
        return tuple(outs)

    devices = jax.devices()[:n_cores]
    mesh = Mesh(np.asarray(devices), ("core",))
    spec = NamedSharding(mesh, PartitionSpec("core"))
    donate = tuple(range(n_params, n_params + len(out_names)))
    sharded = jax.jit(
        shard_map(
            _body,
            mesh=mesh,
            in_specs=(PartitionSpec("core"),) * (n_params + len(out_names)),
            out_specs=(PartitionSpec("core"),) * len(out_names),
            check_rep=False,
        ),
        donate_argnums=donate,
        keep_unused=True,
    )
    din = [
        jax.device_put(
            np.concatenate([m[name] for m in in_maps], axis=0), spec
        )
        for name in in_names
    ]
    zero_shapes = [
        (n_cores * a.shape[0], *a.shape[1:]) for a in out_avals
    ]
    zeros_fn = jax.jit(
        lambda: tuple(
            jnp.zeros(s, a.dtype) for s, a in zip(zero_shapes, out_avals)
        ),
        out_shardings=tuple(spec for _ in out_avals),
    )

    def run_once():
        return sharded(*din, *zeros_fn())

    def zeros_only():
        return zeros_fn()

    return run_once, zeros_only
